# revision 3
# baseline (speedup 1.0000x reference)
"""PoolEdgesToVertices: v_out = v_in + conEd @ e_in, on 8 TRN2 NeuronCores.

Sharding: rows of conEd/v_in (vertex dim V=4096) split across 8 cores
(512 rows each); e_in replicated. adj/u_in are pass-throughs and never
touch the device.

Per-core kernel:
  - e_in resident in SBUF as e_sb[p, t, d] = e_in[t*128+p, d]
  - stream conEd shard as [128v, 2048e] tiles (8 KiB/partition descriptors)
  - each [128v,128e] sub-block: PE-transpose via identity -> PSUM ->
    DVE/ACT copy -> SBUF, then accumulating matmul
    acc[32d,128v] += e_tile[128e,32d].T @ T[128e,128v]
  - final per v-block: PE-transpose acc -> [128v,32d], add v_in, DMA out
"""

import numpy as np

V, E, D = 4096, 32768, 32
N_CORES = 8
P = 128
V_SH = V // N_CORES          # 512 rows per core
VB = V_SH // P               # 4 v-blocks per core
EB = E // P                  # 256 e-blocks
E_TILE = 2048                # conEd DMA tile width (16 e-blocks, 1 MiB)
NT = E // E_TILE             # 16 DMA tiles per v-block
ET_PER = E_TILE // P         # 16 e-blocks per DMA tile

_CACHE = {}


def _ensure_path():
    import sys
    p = "/opt/trn_rl_repo"
    if p not in sys.path:
        sys.path.insert(0, p)


def build_module():
    _ensure_path()
    from contextlib import ExitStack

    import concourse.mybir as mybir
    import concourse.tile as tile
    from concourse import bacc
    from concourse.masks import make_identity

    f32 = mybir.dt.float32
    # Bacc (not plain Bass): its compile() pass splits multi-wait sync_infos
    # (TRN2 allows 1 wait/instruction) — plain Bass trips walrus codegen.
    nc = bacc.Bacc("TRN2", target_bir_lowering=False)

    conEd = nc.dram_tensor("conEd", [V_SH, E], f32, kind="ExternalInput")
    v_in = nc.dram_tensor("v_in", [V_SH, D], f32, kind="ExternalInput")
    e_in = nc.dram_tensor("e_in", [E, D], f32, kind="ExternalInput")
    v_out = nc.dram_tensor("v_out", [V_SH, D], f32, kind="ExternalOutput")

    with tile.TileContext(nc) as tc, ExitStack() as ctx:
        const_pool = ctx.enter_context(tc.tile_pool(name="const", bufs=1))
        e_pool = ctx.enter_context(tc.tile_pool(name="e_res", bufs=1))
        ced_pool = ctx.enter_context(tc.tile_pool(name="ced", bufs=12))
        t_pool = ctx.enter_context(tc.tile_pool(name="tsb", bufs=8))
        pt_pool = ctx.enter_context(tc.tile_pool(name="pt", bufs=4, space="PSUM"))
        acc_pool = ctx.enter_context(tc.tile_pool(name="acc", bufs=2, space="PSUM"))
        fin_pool = ctx.enter_context(tc.tile_pool(name="fin", bufs=1, space="PSUM"))
        out_pool = ctx.enter_context(tc.tile_pool(name="outp", bufs=4))

        ident = const_pool.tile([P, P], f32, name="ident")
        make_identity(nc, ident)

        # Resident replicated e_in: e_sb[p, t, d] = e_in[t*128 + p, d].
        # 128 B contiguous runs; split across 8 dma_starts for queue spread.
        e_sb = e_pool.tile([P, EB, D], f32, name="e_sb")
        e_src = e_in.ap().rearrange("(t p) d -> p t d", p=P)
        n_chunk = 8
        c = EB // n_chunk
        for q in range(n_chunk):
            nc.sync.dma_start(out=e_sb[:, q * c:(q + 1) * c],
                              in_=e_src[:, q * c:(q + 1) * c])

        for vb in range(VB):
            v_tile = out_pool.tile([P, D], f32, name="v_tile", tag="v_tile")
            nc.sync.dma_start(out=v_tile, in_=v_in[vb * P:(vb + 1) * P, :])

            acc = acc_pool.tile([D, 512], f32, name="acc", tag="acc")
            for et in range(NT):
                ced = ced_pool.tile([P, E_TILE], f32, name="ced", tag="ced")
                h = P // 2
                e0 = et * E_TILE
                nc.sync.dma_start(out=ced[:h],
                                  in_=conEd[vb * P:vb * P + h, e0:e0 + E_TILE])
                nc.sync.dma_start(out=ced[h:],
                                  in_=conEd[vb * P + h:(vb + 1) * P, e0:e0 + E_TILE])
                for i in range(ET_PER):
                    eb = et * ET_PER + i
                    pt = pt_pool.tile([P, 512], f32, name="pt", tag="pt")
                    nc.tensor.transpose(pt[:, :P], ced[:, i * P:(i + 1) * P], ident)
                    tsb = t_pool.tile([P, P], f32, name="tsb", tag="tsb")
                    if eb % 2 == 0:
                        nc.vector.tensor_copy(out=tsb, in_=pt[:, :P])
                    else:
                        nc.scalar.copy(out=tsb, in_=pt[:, :P])
                    nc.tensor.matmul(
                        acc[:, :P],
                        e_sb[:, eb],
                        tsb,
                        start=(eb == 0),
                        stop=(eb == EB - 1),
                    )

            # acc[:, :P] = pooled.T  [32d, 128v] -> transpose, add v_in, store
            accs = out_pool.tile([D, P], f32, name="accs", tag="accs")
            nc.vector.tensor_copy(out=accs, in_=acc[:, :P])
            fin = fin_pool.tile([P, 512], f32, name="fin", tag="fin")
            nc.tensor.transpose(fin[:, :D], accs, ident[:D, :D])
            res = out_pool.tile([P, D], f32, name="res", tag="res")
            nc.vector.tensor_add(res, fin[:, :D], v_tile)
            nc.sync.dma_start(out=v_out[vb * P:(vb + 1) * P, :], in_=res)

    nc.finalize()
    return nc


def _run_spmd(v_in, e_in, conEd, trace=False):
    _ensure_path()
    from concourse.bass_utils import run_bass_kernel_spmd

    nc = _CACHE.get("nc")
    if nc is None:
        nc = build_module()
        _CACHE["nc"] = nc

    in_maps = []
    for ci in range(N_CORES):
        in_maps.append({
            "conEd": np.ascontiguousarray(conEd[ci * V_SH:(ci + 1) * V_SH]),
            "v_in": np.ascontiguousarray(v_in[ci * V_SH:(ci + 1) * V_SH]),
            "e_in": np.ascontiguousarray(e_in),
        })
    res = run_bass_kernel_spmd(nc, in_maps, list(range(N_CORES)), trace=trace)
    v_out = np.concatenate(
        [np.asarray(res.results[i]["v_out"]) for i in range(N_CORES)], axis=0
    )
    return v_out, res


def kernel(v_in, e_in, u_in, adj, conEd):
    v_in = np.asarray(v_in, dtype=np.float32)
    e_in_np = np.asarray(e_in, dtype=np.float32)
    conEd_np = np.asarray(conEd, dtype=np.float32)
    v_out, _ = _run_spmd(v_in, e_in_np, conEd_np)
    return (v_out, e_in, u_in, adj, conEd)
